# revision 14
# baseline (speedup 1.0000x reference)
"""Trainium2 Bass kernel: per-channel 256-bin normalized histogram.

Input: full inputs [64, 512, 512, 3] float32 in [0, 1).
Output: [256, 3] float32 — per-channel histogram normalized to sum 1.

Strategy (8 NeuronCores, data-parallel over the batch dim):
  Each core gets 8 batches = 6,291,456 elements laid out [128, 49152]
  (partition p holds 16384 consecutive pixels, channel-interleaved).

  Per core:
    1. Prep (VectorE): exact bin index idx = floor(x*256) via the fp32
       magic-number round ((y + 2^23) - 2^23) plus a compare fix-up,
       stored channel-separated as bf16 [128, 3, 16384] in SBUF.
    2. Count 256 bins x 3 channels, split across three engine routes:
       - PE route (bins 0..NPE-1): VectorE builds the is_equal indicator
         plane at 4x bf16 rate; TensorE reduces it with matmuls whose
         stationary operand is a ones-column window, so bin j's count
         accumulates at PSUM partition j of a per-channel [128, 512]
         PSUM tile.  One tensor_reduce per channel folds the whole bank
         to [128, 1] = 128 bin counts.
       - ScalarE route: activation(Sign, bias=0.5-b, accum) — a CDF;
         counts recovered on host by first differences.
       - GPSIMD route: tensor_scalar(is_equal, accum) on the Q7 cores —
         an independent 4th engine counting its own share of bins.
    3. DMA the small per-partition accumulators to HBM.

  Host: sums accumulators (exact integer counts in fp64), all-reduces the
  8 cores' counts, applies the per-channel fp32 normalization divide.

Counting is exact (integer counts < 2^24 in fp32 accumulators), so the
result matches the reference bit-for-bit up to the final fp32 divide.
"""

import os

import numpy as np

import concourse.bacc as bacc
import concourse.mybir as mybir
from concourse.bass_utils import run_bass_kernel_spmd
from concourse.tile import TileContext

# Problem constants (hardcoded per contract)
B, H, W, C = 64, 512, 512, 3
NBINS = 256
NCORES = 8
P = 128

BPC = B // NCORES                     # 8 batches per core
EPC = BPC * H * W * C                 # 6,291,456 elements per core
ROW = EPC // P                        # 49,152 fp32 per partition
PIXROW = ROW // C                     # 16,384 per channel per partition
CHUNK = 3072
NCHUNK = ROW // CHUNK                 # 16
CPIX = CHUNK // C                     # 1024

# Per-channel bin split across engine routes (sums to 256).
NPE = 160                             # bins [0, NPE) reduced on TensorE
NPE_GP = 60                           # ... of which this many planes are
                                      #     built by GPSIMD (rest on VectorE)
NSC = 76                              # bins [NPE, NPE+NSC) on ScalarE
NDF = NBINS - NPE - NSC               # bins [NPE+NSC, 256) fused on VectorE

QTR = PIXROW // 4                     # PE planes are built in quarters
MMCOL = 512                           # matmul moving-columns per op

MAGIC = float(np.float32(2.0 ** 23))
AL = mybir.AluOpType
AF = mybir.ActivationFunctionType

_CACHE: dict = {}


def _build_module():
    nc = bacc.Bacc("TRN2", target_bir_lowering=False, debug=False,
                   num_devices=NCORES)

    x_ext = nc.declare_dram_parameter("x", [P, ROW], mybir.dt.float32,
                                      isOutput=False)
    bias_ext = nc.declare_dram_parameter("bias_tab", [P, NBINS],
                                         mybir.dt.float32, isOutput=False)
    accp_ext = nc.declare_dram_parameter("acc_pe", [P, 2 * C],
                                         mybir.dt.float32, isOutput=True)
    accs_ext = nc.declare_dram_parameter("acc_sc", [P, C * NSC],
                                         mybir.dt.float32, isOutput=True)
    accd_ext = nc.declare_dram_parameter("acc_df", [P, C * NDF],
                                         mybir.dt.float32, isOutput=True)

    with TileContext(nc) as tc:
        with tc.tile_pool(name="persist", bufs=1) as pp:
            idx = pp.tile([P, C, PIXROW], mybir.dt.bfloat16, tag="idx")
            acc_pe = pp.tile([P, 2 * C], mybir.dt.float32, tag="accp")
            acc_sc = pp.tile([P, C * NSC], mybir.dt.float32, tag="accs")
            acc_df = pp.tile([P, C * NDF], mybir.dt.float32, tag="accd")
            bias_tab = pp.tile([P, NBINS], mybir.dt.float32, tag="bias")
            # ones-column window: zeros [P, 255] with ones in column 127.
            # lhsT = zo[:, 127-j : 255-j] puts the ones at weight column j,
            # so the matmul lands the plane's column-sums on PSUM row j.
            zo = pp.tile([P, 2 * P - 1], mybir.dt.bfloat16, tag="zo")

            nc.sync.dma_start(out=bias_tab[:], in_=bias_ext.ap())
            nc.gpsimd.memset(zo[:], 0.0)
            nc.gpsimd.memset(zo[:, P - 1:P], 1.0)

            # ---- Phase 1: prep ----
            with tc.tile_pool(name="prep", bufs=2) as prep:
                for k in range(NCHUNK):
                    stage = prep.tile([P, CHUNK], mybir.dt.float32,
                                      tag="stage")
                    tsc = prep.tile([P, CHUNK], mybir.dt.float32, tag="tsc")
                    nc.sync.dma_start(
                        out=stage[:],
                        in_=x_ext.ap()[:, k * CHUNK:(k + 1) * CHUNK])
                    # y = min(x*256, 255.5)  (in place)
                    nc.vector.tensor_scalar(
                        stage[:], stage[:], 256.0, 255.5, AL.mult, AL.min)
                    # t = (y + M) - M : round-to-nearest-even integer
                    nc.vector.tensor_scalar(
                        tsc[:], stage[:], MAGIC, -MAGIC, AL.add, AL.add)
                    # g = t > y  (overwrites y in place)
                    nc.vector.scalar_tensor_tensor(
                        stage[:], tsc[:], 0.0, stage[:], AL.bypass, AL.is_gt)
                    # idx_c = t - g, channel-split, bf16
                    for c in range(C):
                        nc.vector.scalar_tensor_tensor(
                            idx[:, c, k * CPIX:(k + 1) * CPIX],
                            stage[:, c::C], -1.0, tsc[:, c::C],
                            AL.mult, AL.add)

            # ---- Phase 2: count passes, three routes ----
            with (tc.tile_pool(name="planes", bufs=3) as plp,
                  tc.tile_pool(name="gplanes", bufs=2) as gplp,
                  tc.tile_pool(name="sscr", bufs=1) as ssp,
                  tc.tile_pool(name="psum", bufs=2, space="PSUM") as psp):
                for c in range(C):
                    ps0 = psp.tile([P, MMCOL], mybir.dt.float32, tag="ps0")
                    ps1 = psp.tile([P, MMCOL], mybir.dt.float32, tag="ps1")
                    ps = [ps0, ps1]

                    # interleaved emission keeps all four queues fed.
                    # GPSIMD-created planes are spread through the PE bins.
                    gp_bins = set(
                        np.linspace(0, NPE - 1, NPE_GP).astype(int).tolist())
                    sched = []
                    si = NPE + 0
                    di = NPE + NSC
                    for b in range(NPE):
                        sched.append(("pe", b))
                        if b % 2 == 0 and si < NPE + NSC:
                            sched.append(("sc", si)); si += 1
                        if b % 8 == 1 and di < NBINS:
                            sched.append(("df", di)); di += 1
                    while si < NPE + NSC:
                        sched.append(("sc", si)); si += 1
                    while di < NBINS:
                        sched.append(("df", di)); di += 1

                    mm_done = [0, 0]
                    n_mm = [min(NPE, P) * 4 * (QTR // MMCOL),
                            max(NPE - P, 0) * 4 * (QTR // MMCOL)]
                    for route, b in sched:
                        if route == "pe":
                            bank, brow = (0, b) if b < P else (1, b - P)
                            w = zo[:, P - 1 - brow:2 * P - 1 - brow]
                            on_gp = b in gp_bins
                            for h in range(4):
                                if on_gp:
                                    plane = gplp.tile([P, QTR],
                                                      mybir.dt.bfloat16,
                                                      tag="gplane")
                                    nc.gpsimd.tensor_scalar(
                                        plane[:],
                                        idx[:, c, h * QTR:(h + 1) * QTR],
                                        float(b), None, AL.is_equal)
                                else:
                                    plane = plp.tile([P, QTR],
                                                     mybir.dt.bfloat16,
                                                     tag="plane")
                                    nc.vector.tensor_scalar(
                                        plane[:],
                                        idx[:, c, h * QTR:(h + 1) * QTR],
                                        float(b), None, AL.is_equal)
                                for j in range(QTR // MMCOL):
                                    nc.tensor.matmul(
                                        ps[bank][:], w,
                                        plane[:, j * MMCOL:(j + 1) * MMCOL],
                                        start=(mm_done[bank] == 0),
                                        stop=(mm_done[bank]
                                              == n_mm[bank] - 1))
                                    mm_done[bank] += 1
                        elif route == "sc":
                            col = c * NSC + (b - NPE)
                            scr = ssp.tile([P, PIXROW], mybir.dt.bfloat16,
                                           tag="s")
                            nc.scalar.activation(
                                scr[:], idx[:, c, :], AF.Sign,
                                bias=bias_tab[:, b:b + 1], scale=1.0,
                                accum_out=acc_sc[:, col:col + 1])
                        else:
                            col = c * NDF + (b - NPE - NSC)
                            scr = ssp.tile([P, PIXROW], mybir.dt.bfloat16,
                                           tag="d")
                            nc.vector.tensor_scalar(
                                scr[:], idx[:, c, :], float(b), None,
                                AL.is_equal, AL.add,
                                accum_out=acc_df[:, col:col + 1])

                    # fold the channel's PSUM banks: bank row j = bin count
                    nc.vector.tensor_reduce(
                        acc_pe[:, 2 * c:2 * c + 1], ps[0][:],
                        mybir.AxisListType.X, AL.add)
                    nc.vector.tensor_reduce(
                        acc_pe[:, 2 * c + 1:2 * c + 2], ps[1][:],
                        mybir.AxisListType.X, AL.add)

            # ---- Phase 3: results out ----
            nc.sync.dma_start(out=accp_ext.ap(), in_=acc_pe[:])
            nc.sync.dma_start(out=accs_ext.ap(), in_=acc_sc[:])
            nc.sync.dma_start(out=accd_ext.ap(), in_=acc_df[:])

    nc.finalize()
    return nc


def _get_module():
    if "nc" not in _CACHE:
        _CACHE["nc"] = _build_module()
    return _CACHE["nc"]


def _decode_counts(results):
    counts = np.zeros((C, NBINS), dtype=np.float64)
    s_tot = np.zeros((C, NSC), dtype=np.float64)
    for r in results:
        ap = r["acc_pe"].astype(np.float64)          # [P, 2C]: row j = bin j
        asc = r["acc_sc"].astype(np.float64)
        ad = r["acc_df"].astype(np.float64)
        for c in range(C):
            counts[c, :P] += ap[:, 2 * c]
            counts[c, P:NPE] += ap[:NPE - P, 2 * c + 1]
        s_tot += asc.sum(axis=0).reshape(C, NSC)
        counts[:, NPE + NSC:] += ad.sum(axis=0).reshape(C, NDF)
    # Sign sums -> CDF: A[b] = 2*#{idx>=b} - TOT
    tot = float(NCORES * P * PIXROW)
    s_ge = (s_tot + tot) / 2.0                        # S_ge(b), b=NPE..NPE+NSC-1
    # S_ge(NPE+NSC) = sum of the DF-route counts
    s_end = counts[:, NPE + NSC:].sum(axis=1, keepdims=True)
    diff = np.empty((C, NSC), dtype=np.float64)
    diff[:, :-1] = s_ge[:, :-1] - s_ge[:, 1:]
    diff[:, -1:] = s_ge[:, -1:] - s_end
    counts[:, NPE:NPE + NSC] = diff
    return counts


def run(x: np.ndarray, trace: bool = False):
    nc = _get_module()

    x = np.ascontiguousarray(x, dtype=np.float32)
    assert x.shape == (B, H, W, C)
    shards = x.reshape(NCORES, P, ROW)

    bias_tab = np.tile((0.5 - np.arange(NBINS, dtype=np.float32))[None, :],
                       (P, 1))
    in_maps = [{"x": shards[i], "bias_tab": bias_tab} for i in range(NCORES)]

    res = run_bass_kernel_spmd(nc, in_maps, list(range(NCORES)), trace=trace)

    counts = _decode_counts(res.results)
    # Normalization exactly as the reference: fp32 divide, then transpose.
    counts32 = counts.astype(np.float32)
    sums = counts32.sum(axis=1, keepdims=True, dtype=np.float32)
    hist = counts32 / sums
    return np.ascontiguousarray(hist.T), res


def kernel(**inputs) -> np.ndarray:
    out, _ = run(inputs["inputs"],
                 trace=bool(os.environ.get("KERNEL_TRACE")))
    return out


# revision 15
# speedup vs baseline: 10.2090x; 10.2090x over previous
"""Trainium2 Bass kernel: per-channel 256-bin normalized histogram.

Input: full inputs [64, 512, 512, 3] float32 in [0, 1).
Output: [256, 3] float32 — per-channel histogram normalized to sum 1.

Strategy (8 NeuronCores, data-parallel over the batch dim):
  Each core gets 8 batches = 6,291,456 elements laid out [128, 49152]
  (partition p holds 16384 consecutive pixels, channel-interleaved).

  Per core:
    1. Prep (VectorE): exact bin index idx = floor(x*256) via the fp32
       magic-number round ((y + 2^23) - 2^23) plus a compare fix-up,
       stored channel-separated as bf16 [128, 3, 16384] in SBUF.
    2. Count 256 bins x 3 channels, split across three engine routes:
       - PE route (bins 0..NPE-1): VectorE builds the is_equal indicator
         plane at 4x bf16 rate; TensorE reduces it with matmuls whose
         stationary operand is a ones-column window, so bin j's count
         accumulates at PSUM partition j of a per-channel [128, 512]
         PSUM tile.  One tensor_reduce per channel folds the whole bank
         to [128, 1] = 128 bin counts.
       - ScalarE route: activation(Sign, bias=0.5-b, accum) — a CDF;
         counts recovered on host by first differences.
       - GPSIMD route: tensor_scalar(is_equal, accum) on the Q7 cores —
         an independent 4th engine counting its own share of bins.
    3. DMA the small per-partition accumulators to HBM.

  Host: sums accumulators (exact integer counts in fp64), all-reduces the
  8 cores' counts, applies the per-channel fp32 normalization divide.

Counting is exact (integer counts < 2^24 in fp32 accumulators), so the
result matches the reference bit-for-bit up to the final fp32 divide.
"""

import os

import numpy as np

import concourse.bacc as bacc
import concourse.mybir as mybir
from concourse.bass_utils import run_bass_kernel_spmd
from concourse.tile import TileContext

# Problem constants (hardcoded per contract)
B, H, W, C = 64, 512, 512, 3
NBINS = 256
NCORES = 8
P = 128

BPC = B // NCORES                     # 8 batches per core
EPC = BPC * H * W * C                 # 6,291,456 elements per core
ROW = EPC // P                        # 49,152 fp32 per partition
PIXROW = ROW // C                     # 16,384 per channel per partition
CHUNK = 3072
NCHUNK = ROW // CHUNK                 # 16
CPIX = CHUNK // C                     # 1024

# Per-channel bin split across engine routes (sums to 256).
NPE = 144                             # bins [0, NPE) reduced on TensorE
NPE_GP = 0                            # ... of which this many planes are
                                      #     built by GPSIMD (rest on VectorE)
NSC = 84                              # bins [NPE, NPE+NSC) on ScalarE
NDF = NBINS - NPE - NSC               # bins [NPE+NSC, 256) fused on VectorE

QTR = PIXROW // 4                     # PE planes are built in quarters
MMCOL = 512                           # matmul moving-columns per op

MAGIC = float(np.float32(2.0 ** 23))
AL = mybir.AluOpType
AF = mybir.ActivationFunctionType

_CACHE: dict = {}


def _build_module():
    nc = bacc.Bacc("TRN2", target_bir_lowering=False, debug=False,
                   num_devices=NCORES)

    x_ext = nc.declare_dram_parameter("x", [P, ROW], mybir.dt.float32,
                                      isOutput=False)
    bias_ext = nc.declare_dram_parameter("bias_tab", [P, NBINS],
                                         mybir.dt.float32, isOutput=False)
    accp_ext = nc.declare_dram_parameter("acc_pe", [P, 2 * C],
                                         mybir.dt.float32, isOutput=True)
    accs_ext = nc.declare_dram_parameter("acc_sc", [P, C * NSC],
                                         mybir.dt.float32, isOutput=True)
    accd_ext = nc.declare_dram_parameter("acc_df", [P, C * NDF],
                                         mybir.dt.float32, isOutput=True)

    with TileContext(nc) as tc:
        with tc.tile_pool(name="persist", bufs=1) as pp:
            idx = pp.tile([P, C, PIXROW], mybir.dt.bfloat16, tag="idx")
            acc_pe = pp.tile([P, 2 * C], mybir.dt.float32, tag="accp")
            acc_sc = pp.tile([P, C * NSC], mybir.dt.float32, tag="accs")
            acc_df = pp.tile([P, C * NDF], mybir.dt.float32, tag="accd")
            bias_tab = pp.tile([P, NBINS], mybir.dt.float32, tag="bias")
            # ones-column window: zeros [P, 255] with ones in column 127.
            # lhsT = zo[:, 127-j : 255-j] puts the ones at weight column j,
            # so the matmul lands the plane's column-sums on PSUM row j.
            zo = pp.tile([P, 2 * P - 1], mybir.dt.bfloat16, tag="zo")

            nc.sync.dma_start(out=bias_tab[:], in_=bias_ext.ap())
            nc.gpsimd.memset(zo[:], 0.0)
            nc.gpsimd.memset(zo[:, P - 1:P], 1.0)

            # ---- Phase 1: prep ----
            with tc.tile_pool(name="prep", bufs=2) as prep:
                for k in range(NCHUNK):
                    stage = prep.tile([P, CHUNK], mybir.dt.float32,
                                      tag="stage")
                    tsc = prep.tile([P, CHUNK], mybir.dt.float32, tag="tsc")
                    nc.sync.dma_start(
                        out=stage[:],
                        in_=x_ext.ap()[:, k * CHUNK:(k + 1) * CHUNK])
                    # y = min(x*256, 255.5)  (in place)
                    nc.vector.tensor_scalar(
                        stage[:], stage[:], 256.0, 255.5, AL.mult, AL.min)
                    # t = (y + M) - M : round-to-nearest-even integer
                    nc.vector.tensor_scalar(
                        tsc[:], stage[:], MAGIC, -MAGIC, AL.add, AL.add)
                    # g = t > y  (overwrites y in place)
                    nc.vector.scalar_tensor_tensor(
                        stage[:], tsc[:], 0.0, stage[:], AL.bypass, AL.is_gt)
                    # idx_c = t - g, channel-split, bf16
                    for c in range(C):
                        nc.vector.scalar_tensor_tensor(
                            idx[:, c, k * CPIX:(k + 1) * CPIX],
                            stage[:, c::C], -1.0, tsc[:, c::C],
                            AL.mult, AL.add)

            # ---- Phase 2: count passes, three routes ----
            with (tc.tile_pool(name="planes", bufs=3) as plp,
                  tc.tile_pool(name="gplanes", bufs=2) as gplp,
                  tc.tile_pool(name="sscr", bufs=1) as ssp,
                  tc.tile_pool(name="psum", bufs=2, space="PSUM") as psp):
                for c in range(C):
                    ps0 = psp.tile([P, MMCOL], mybir.dt.float32, tag="ps0")
                    ps1 = psp.tile([P, MMCOL], mybir.dt.float32, tag="ps1")
                    ps = [ps0, ps1]

                    # interleaved emission keeps all four queues fed.
                    # GPSIMD-created planes are spread through the PE bins.
                    gp_bins = set(
                        np.linspace(0, NPE - 1, NPE_GP).astype(int).tolist())
                    sched = []
                    si = NPE + 0
                    di = NPE + NSC
                    for b in range(NPE):
                        sched.append(("pe", b))
                        if b % 2 == 0 and si < NPE + NSC:
                            sched.append(("sc", si)); si += 1
                        if b % 8 == 1 and di < NBINS:
                            sched.append(("df", di)); di += 1
                    while si < NPE + NSC:
                        sched.append(("sc", si)); si += 1
                    while di < NBINS:
                        sched.append(("df", di)); di += 1

                    mm_done = [0, 0]
                    n_mm = [min(NPE, P) * 4 * (QTR // MMCOL),
                            max(NPE - P, 0) * 4 * (QTR // MMCOL)]
                    for route, b in sched:
                        if route == "pe":
                            bank, brow = (0, b) if b < P else (1, b - P)
                            w = zo[:, P - 1 - brow:2 * P - 1 - brow]
                            on_gp = b in gp_bins
                            for h in range(4):
                                if on_gp:
                                    plane = gplp.tile([P, QTR],
                                                      mybir.dt.bfloat16,
                                                      tag="gplane")
                                    nc.gpsimd.tensor_scalar(
                                        plane[:],
                                        idx[:, c, h * QTR:(h + 1) * QTR],
                                        float(b), None, AL.is_equal)
                                else:
                                    plane = plp.tile([P, QTR],
                                                     mybir.dt.bfloat16,
                                                     tag="plane")
                                    nc.vector.tensor_scalar(
                                        plane[:],
                                        idx[:, c, h * QTR:(h + 1) * QTR],
                                        float(b), None, AL.is_equal)
                                for j in range(QTR // MMCOL):
                                    nc.tensor.matmul(
                                        ps[bank][:], w,
                                        plane[:, j * MMCOL:(j + 1) * MMCOL],
                                        start=(mm_done[bank] == 0),
                                        stop=(mm_done[bank]
                                              == n_mm[bank] - 1))
                                    mm_done[bank] += 1
                        elif route == "sc":
                            col = c * NSC + (b - NPE)
                            scr = ssp.tile([P, PIXROW], mybir.dt.bfloat16,
                                           tag="s")
                            nc.scalar.activation(
                                scr[:], idx[:, c, :], AF.Sign,
                                bias=bias_tab[:, b:b + 1], scale=1.0,
                                accum_out=acc_sc[:, col:col + 1])
                        else:
                            col = c * NDF + (b - NPE - NSC)
                            scr = ssp.tile([P, PIXROW], mybir.dt.bfloat16,
                                           tag="d")
                            nc.vector.tensor_scalar(
                                scr[:], idx[:, c, :], float(b), None,
                                AL.is_equal, AL.add,
                                accum_out=acc_df[:, col:col + 1])

                    # fold the channel's PSUM banks: bank row j = bin count
                    nc.vector.tensor_reduce(
                        acc_pe[:, 2 * c:2 * c + 1], ps[0][:],
                        mybir.AxisListType.X, AL.add)
                    nc.vector.tensor_reduce(
                        acc_pe[:, 2 * c + 1:2 * c + 2], ps[1][:],
                        mybir.AxisListType.X, AL.add)

            # ---- Phase 3: results out ----
            nc.sync.dma_start(out=accp_ext.ap(), in_=acc_pe[:])
            nc.sync.dma_start(out=accs_ext.ap(), in_=acc_sc[:])
            nc.sync.dma_start(out=accd_ext.ap(), in_=acc_df[:])

    nc.finalize()
    return nc


def _get_module():
    if "nc" not in _CACHE:
        _CACHE["nc"] = _build_module()
    return _CACHE["nc"]


def _decode_counts(results):
    counts = np.zeros((C, NBINS), dtype=np.float64)
    s_tot = np.zeros((C, NSC), dtype=np.float64)
    for r in results:
        ap = r["acc_pe"].astype(np.float64)          # [P, 2C]: row j = bin j
        asc = r["acc_sc"].astype(np.float64)
        ad = r["acc_df"].astype(np.float64)
        for c in range(C):
            counts[c, :P] += ap[:, 2 * c]
            counts[c, P:NPE] += ap[:NPE - P, 2 * c + 1]
        s_tot += asc.sum(axis=0).reshape(C, NSC)
        counts[:, NPE + NSC:] += ad.sum(axis=0).reshape(C, NDF)
    # Sign sums -> CDF: A[b] = 2*#{idx>=b} - TOT
    tot = float(NCORES * P * PIXROW)
    s_ge = (s_tot + tot) / 2.0                        # S_ge(b), b=NPE..NPE+NSC-1
    # S_ge(NPE+NSC) = sum of the DF-route counts
    s_end = counts[:, NPE + NSC:].sum(axis=1, keepdims=True)
    diff = np.empty((C, NSC), dtype=np.float64)
    diff[:, :-1] = s_ge[:, :-1] - s_ge[:, 1:]
    diff[:, -1:] = s_ge[:, -1:] - s_end
    counts[:, NPE:NPE + NSC] = diff
    return counts


def run(x: np.ndarray, trace: bool = False):
    nc = _get_module()

    x = np.ascontiguousarray(x, dtype=np.float32)
    assert x.shape == (B, H, W, C)
    shards = x.reshape(NCORES, P, ROW)

    bias_tab = np.tile((0.5 - np.arange(NBINS, dtype=np.float32))[None, :],
                       (P, 1))
    in_maps = [{"x": shards[i], "bias_tab": bias_tab} for i in range(NCORES)]

    res = run_bass_kernel_spmd(nc, in_maps, list(range(NCORES)), trace=trace)

    counts = _decode_counts(res.results)
    # Normalization exactly as the reference: fp32 divide, then transpose.
    counts32 = counts.astype(np.float32)
    sums = counts32.sum(axis=1, keepdims=True, dtype=np.float32)
    hist = counts32 / sums
    return np.ascontiguousarray(hist.T), res


def kernel(**inputs) -> np.ndarray:
    out, _ = run(inputs["inputs"],
                 trace=bool(os.environ.get("KERNEL_TRACE")))
    return out


# revision 21
# speedup vs baseline: 11.0158x; 1.0790x over previous
"""Trainium2 Bass kernel: per-channel 256-bin normalized histogram.

Input: full inputs [64, 512, 512, 3] float32 in [0, 1).
Output: [256, 3] float32 — per-channel histogram normalized to sum 1.

Strategy (8 NeuronCores, data-parallel over the batch dim):
  Each core gets 8 batches = 6,291,456 elements laid out [128, 49152]
  (partition p holds 16384 consecutive pixels, channel-interleaved).

  Per core:
    1. Prep (VectorE): exact bin index idx = floor(x*256) via the fp32
       magic-number round ((y + 2^23) - 2^23) plus a compare fix-up,
       stored channel-separated as bf16 [128, 3, 16384] in SBUF.
    2. Count 256 bins x 3 channels, split across three engine routes:
       - PE route (bins 0..NPE-1): VectorE builds the is_equal indicator
         plane at 4x bf16 rate; TensorE reduces it with matmuls whose
         stationary operand is a ones-column window, so bin j's count
         accumulates at PSUM partition j of a per-channel [128, 512]
         PSUM tile.  One tensor_reduce per channel folds the whole bank
         to [128, 1] = 128 bin counts.
       - ScalarE route: activation(Sign, bias=0.5-b, accum) — a CDF;
         counts recovered on host by first differences.
       - GPSIMD route: tensor_scalar(is_equal, accum) on the Q7 cores —
         an independent 4th engine counting its own share of bins.
    3. DMA the small per-partition accumulators to HBM.

  Host: sums accumulators (exact integer counts in fp64), all-reduces the
  8 cores' counts, applies the per-channel fp32 normalization divide.

Counting is exact (integer counts < 2^24 in fp32 accumulators), so the
result matches the reference bit-for-bit up to the final fp32 divide.
"""

import os

import numpy as np

import concourse.bacc as bacc
import concourse.mybir as mybir
from concourse.bass_utils import run_bass_kernel_spmd
from concourse.tile import TileContext

# Problem constants (hardcoded per contract)
B, H, W, C = 64, 512, 512, 3
NBINS = 256
NCORES = 8
P = 128

BPC = B // NCORES                     # 8 batches per core
EPC = BPC * H * W * C                 # 6,291,456 elements per core
ROW = EPC // P                        # 49,152 fp32 per partition
PIXROW = ROW // C                     # 16,384 per channel per partition
CHUNK = 3072
NCHUNK = ROW // CHUNK                 # 16
CPIX = CHUNK // C                     # 1024

# Per-channel bin split across engine routes (sums to 256).
NPE = 155                             # bins [0, NPE) reduced on TensorE
NPE_GP = 0                            # ... of which this many planes are
                                      #     built by GPSIMD (rest on VectorE)
NSC = 80                              # bins [NPE, NPE+NSC) on ScalarE
NDF = NBINS - NPE - NSC               # bins [NPE+NSC, 256) fused on VectorE

QTR = PIXROW // 4                     # PE planes are built in quarters
MMCOL = 512                           # matmul moving-columns per op

MAGIC = float(np.float32(2.0 ** 23))
AL = mybir.AluOpType
AF = mybir.ActivationFunctionType

_CACHE: dict = {}


def _build_module():
    nc = bacc.Bacc("TRN2", target_bir_lowering=False, debug=False,
                   num_devices=NCORES)

    x_ext = nc.declare_dram_parameter("x", [P, ROW], mybir.dt.float32,
                                      isOutput=False)
    bias_ext = nc.declare_dram_parameter("bias_tab", [P, NBINS],
                                         mybir.dt.float32, isOutput=False)
    accp_ext = nc.declare_dram_parameter("acc_pe", [P, 2 * C],
                                         mybir.dt.float32, isOutput=True)
    accs_ext = nc.declare_dram_parameter("acc_sc", [P, C * NSC],
                                         mybir.dt.float32, isOutput=True)
    accd_ext = nc.declare_dram_parameter("acc_df", [P, C * NDF * 4],
                                         mybir.dt.float32, isOutput=True)

    with TileContext(nc) as tc:
        with tc.tile_pool(name="persist", bufs=1) as pp:
            idx = pp.tile([P, C, PIXROW], mybir.dt.bfloat16, tag="idx")
            acc_pe = pp.tile([P, 2 * C], mybir.dt.float32, tag="accp")
            acc_sc = pp.tile([P, C * NSC], mybir.dt.float32, tag="accs")
            acc_df = pp.tile([P, C * NDF * 4], mybir.dt.float32, tag="accd")
            bias_tab = pp.tile([P, NBINS], mybir.dt.float32, tag="bias")
            # ones-column window: zeros [P, 255] with ones in column 127.
            # lhsT = zo[:, 127-j : 255-j] puts the ones at weight column j,
            # so the matmul lands the plane's column-sums on PSUM row j.
            zo = pp.tile([P, 2 * P - 1], mybir.dt.bfloat16, tag="zo")

            nc.sync.dma_start(out=bias_tab[:], in_=bias_ext.ap())
            nc.gpsimd.memset(zo[:], 0.0)
            nc.gpsimd.memset(zo[:, P - 1:P], 1.0)

            # ---- Phase 1: prep ----
            with tc.tile_pool(name="prep", bufs=2) as prep:
                for k in range(NCHUNK):
                    stage = prep.tile([P, CHUNK], mybir.dt.float32,
                                      tag="stage")
                    tsc = prep.tile([P, CHUNK], mybir.dt.float32, tag="tsc")
                    nc.sync.dma_start(
                        out=stage[:],
                        in_=x_ext.ap()[:, k * CHUNK:(k + 1) * CHUNK])
                    # y = min(x*256, 255.5)  (in place)
                    nc.vector.tensor_scalar(
                        stage[:], stage[:], 256.0, 255.5, AL.mult, AL.min)
                    # t = (y + M) - M : round-to-nearest-even integer
                    nc.vector.tensor_scalar(
                        tsc[:], stage[:], MAGIC, -MAGIC, AL.add, AL.add)
                    # g = t > y  (overwrites y in place)
                    nc.vector.scalar_tensor_tensor(
                        stage[:], tsc[:], 0.0, stage[:], AL.bypass, AL.is_gt)
                    # idx_c = t - g, channel-split, bf16
                    for c in range(C):
                        nc.vector.scalar_tensor_tensor(
                            idx[:, c, k * CPIX:(k + 1) * CPIX],
                            stage[:, c::C], -1.0, tsc[:, c::C],
                            AL.mult, AL.add)

            # ---- Phase 2: count passes, three routes ----
            with (tc.tile_pool(name="planes", bufs=4) as plp,
                  tc.tile_pool(name="sscr", bufs=1) as ssp,
                  tc.tile_pool(name="psum", bufs=2, space="PSUM") as psp):
                for c in range(C):
                    ps0 = psp.tile([P, MMCOL], mybir.dt.float32, tag="ps0")
                    ps1 = psp.tile([P, MMCOL], mybir.dt.float32, tag="ps1")
                    ps = [ps0, ps1]

                    # interleaved emission keeps all queues fed.  DF work is
                    # split into quarter-row accums so no single DVE op
                    # stalls plane creation longer than a PE quarter-burst.
                    sched = []
                    si = NPE + 0
                    dfq = [(b, q) for b in range(NPE + NSC, NBINS)
                           for q in range(4)]
                    di = 0
                    for b in range(NPE):
                        sched.append(("pe", b, 0))
                        if b % 2 == 0 and si < NPE + NSC:
                            sched.append(("sc", si, 0)); si += 1
                        while di < len(dfq) and di * NPE < b * len(dfq):
                            sched.append(("df",) + dfq[di]); di += 1
                    while si < NPE + NSC:
                        sched.append(("sc", si, 0)); si += 1
                    while di < len(dfq):
                        sched.append(("df",) + dfq[di]); di += 1

                    mm_done = [0, 0]
                    n_mm = [min(NPE, P) * 4 * (QTR // MMCOL),
                            max(NPE - P, 0) * 4 * (QTR // MMCOL)]
                    for route, b, q in sched:
                        if route == "pe":
                            bank, brow = (0, b) if b < P else (1, b - P)
                            w = zo[:, P - 1 - brow:2 * P - 1 - brow]
                            for h in range(4):
                                plane = plp.tile([P, QTR],
                                                 mybir.dt.bfloat16,
                                                 tag="plane")
                                nc.vector.tensor_scalar(
                                    plane[:],
                                    idx[:, c, h * QTR:(h + 1) * QTR],
                                    float(b), None, AL.is_equal)
                                for j in range(QTR // MMCOL):
                                    nc.tensor.matmul(
                                        ps[bank][:], w,
                                        plane[:, j * MMCOL:(j + 1) * MMCOL],
                                        start=(mm_done[bank] == 0),
                                        stop=(mm_done[bank]
                                              == n_mm[bank] - 1))
                                    mm_done[bank] += 1
                        elif route == "sc":
                            col = c * NSC + (b - NPE)
                            scr = ssp.tile([P, PIXROW], mybir.dt.bfloat16,
                                           tag="s")
                            nc.scalar.activation(
                                scr[:], idx[:, c, :], AF.Sign,
                                bias=bias_tab[:, b:b + 1], scale=1.0,
                                accum_out=acc_sc[:, col:col + 1])
                        else:
                            col = 4 * (c * NDF + (b - NPE - NSC)) + q
                            scr = ssp.tile([P, QTR], mybir.dt.bfloat16,
                                           tag="d")
                            nc.vector.tensor_scalar(
                                scr[:], idx[:, c, q * QTR:(q + 1) * QTR],
                                float(b), None, AL.is_equal, AL.add,
                                accum_out=acc_df[:, col:col + 1])

                    # fold the channel's PSUM banks: bank row j = bin count
                    nc.vector.tensor_reduce(
                        acc_pe[:, 2 * c:2 * c + 1], ps[0][:],
                        mybir.AxisListType.X, AL.add)
                    nc.vector.tensor_reduce(
                        acc_pe[:, 2 * c + 1:2 * c + 2], ps[1][:],
                        mybir.AxisListType.X, AL.add)

            # ---- Phase 3: results out ----
            nc.sync.dma_start(out=accp_ext.ap(), in_=acc_pe[:])
            nc.sync.dma_start(out=accs_ext.ap(), in_=acc_sc[:])
            nc.sync.dma_start(out=accd_ext.ap(), in_=acc_df[:])

    nc.finalize()
    return nc


def _get_module():
    if "nc" not in _CACHE:
        _CACHE["nc"] = _build_module()
    return _CACHE["nc"]


def _decode_counts(results):
    counts = np.zeros((C, NBINS), dtype=np.float64)
    s_tot = np.zeros((C, NSC), dtype=np.float64)
    for r in results:
        ap = r["acc_pe"].astype(np.float64)          # [P, 2C]: row j = bin j
        asc = r["acc_sc"].astype(np.float64)
        ad = r["acc_df"].astype(np.float64)
        for c in range(C):
            counts[c, :P] += ap[:, 2 * c]
            counts[c, P:NPE] += ap[:NPE - P, 2 * c + 1]
        s_tot += asc.sum(axis=0).reshape(C, NSC)
        counts[:, NPE + NSC:] += ad.sum(axis=0).reshape(C, NDF, 4).sum(axis=2)
    # Sign sums -> CDF: A[b] = 2*#{idx>=b} - TOT
    tot = float(NCORES * P * PIXROW)
    s_ge = (s_tot + tot) / 2.0                        # S_ge(b), b=NPE..NPE+NSC-1
    # S_ge(NPE+NSC) = sum of the DF-route counts
    s_end = counts[:, NPE + NSC:].sum(axis=1, keepdims=True)
    diff = np.empty((C, NSC), dtype=np.float64)
    diff[:, :-1] = s_ge[:, :-1] - s_ge[:, 1:]
    diff[:, -1:] = s_ge[:, -1:] - s_end
    counts[:, NPE:NPE + NSC] = diff
    return counts


def run(x: np.ndarray, trace: bool = False):
    nc = _get_module()

    x = np.ascontiguousarray(x, dtype=np.float32)
    assert x.shape == (B, H, W, C)
    shards = x.reshape(NCORES, P, ROW)

    bias_tab = np.tile((0.5 - np.arange(NBINS, dtype=np.float32))[None, :],
                       (P, 1))
    in_maps = [{"x": shards[i], "bias_tab": bias_tab} for i in range(NCORES)]

    res = run_bass_kernel_spmd(nc, in_maps, list(range(NCORES)), trace=trace)

    counts = _decode_counts(res.results)
    # Normalization exactly as the reference: fp32 divide, then transpose.
    counts32 = counts.astype(np.float32)
    sums = counts32.sum(axis=1, keepdims=True, dtype=np.float32)
    hist = counts32 / sums
    return np.ascontiguousarray(hist.T), res


def kernel(**inputs) -> np.ndarray:
    out, _ = run(inputs["inputs"],
                 trace=bool(os.environ.get("KERNEL_TRACE")))
    return out


# revision 22
# speedup vs baseline: 11.2998x; 1.0258x over previous
"""Trainium2 Bass kernel: per-channel 256-bin normalized histogram.

Input: full inputs [64, 512, 512, 3] float32 in [0, 1).
Output: [256, 3] float32 — per-channel histogram normalized to sum 1.

Strategy (8 NeuronCores, data-parallel over the batch dim):
  Each core gets 8 batches = 6,291,456 elements laid out [128, 49152]
  (partition p holds 16384 consecutive pixels, channel-interleaved).

  Key trick: bin(x) = floor(x*256) is EXACTLY determined by the
  round-toward-zero bf16 truncation of x (for x in [0,1), the integer
  part of x*256 needs at most the top 7 mantissa bits).  So prep is a
  pure byte-level copy: the high int16 half of each fp32 word,
  de-interleaved per channel (VectorE strided copies, no arithmetic).

  Counting is CDF-based: every route computes S(b) = #{x >= b/256};
  counts are recovered on the host as count[b] = S(b) - S(b+1), exact
  in integer arithmetic.  Three engine routes per channel:
    - PE route (bins [0, NPE)): VectorE builds the is_ge indicator
      plane at 4x bf16 rate; TensorE reduces it with matmuls whose
      stationary operand is a ones-column window, so bin j's S lands
      at PSUM partition j of a per-channel [128, 512] PSUM bank.  One
      tensor_reduce per bank folds it to [128, 1].
    - ScalarE route: activation(Sign, bias=eps_b - b/256, accum) over
      the raw truncated values — a CDF via sign sums.
    - DF route (VectorE fused): tensor_scalar(is_ge, accum) over
      quarter rows, interleaved so plane creation never stalls long.

  Host: sums accumulators (exact integer counts in fp64), all-reduces
  the 8 cores' counts, applies the per-channel fp32 normalize divide.

Counting is exact (integer counts < 2^24 in fp32 accumulators), so the
result matches the reference bit-for-bit up to the final fp32 divide.
"""

import os

import numpy as np

import concourse.bacc as bacc
import concourse.mybir as mybir
from concourse.bass_utils import run_bass_kernel_spmd
from concourse.tile import TileContext

# Problem constants (hardcoded per contract)
B, H, W, C = 64, 512, 512, 3
NBINS = 256
NCORES = 8
P = 128

BPC = B // NCORES                     # 8 batches per core
EPC = BPC * H * W * C                 # 6,291,456 elements per core
ROW = EPC // P                        # 49,152 fp32 per partition
PIXROW = ROW // C                     # 16,384 per channel per partition
CHUNK = 3072
NCHUNK = ROW // CHUNK                 # 16
CPIX = CHUNK // C                     # 1024

# Per-channel bin split across engine routes (sums to 256).
NPE = 155                             # bins [0, NPE) reduced on TensorE
NSC = 80                              # bins [NPE, NPE+NSC) on ScalarE
NDF = NBINS - NPE - NSC               # bins [NPE+NSC, 256) fused on VectorE

QTR = PIXROW // 4                     # PE planes are built in quarters
MMCOL = 512                           # matmul moving-columns per op

AL = mybir.AluOpType
AF = mybir.ActivationFunctionType
I16 = mybir.dt.int16

_CACHE: dict = {}


def _thresh(b: int) -> float:
    return float(np.float32(b / 256.0))


def _build_module():
    nc = bacc.Bacc("TRN2", target_bir_lowering=False, debug=False,
                   num_devices=NCORES)

    x_ext = nc.declare_dram_parameter("x", [P, ROW], mybir.dt.float32,
                                      isOutput=False)
    bias_ext = nc.declare_dram_parameter("bias_tab", [P, NBINS],
                                         mybir.dt.float32, isOutput=False)
    accp_ext = nc.declare_dram_parameter("acc_pe", [P, 2 * C],
                                         mybir.dt.float32, isOutput=True)
    accs_ext = nc.declare_dram_parameter("acc_sc", [P, C * NSC],
                                         mybir.dt.float32, isOutput=True)
    accd_ext = nc.declare_dram_parameter("acc_df", [P, C * NDF * 4],
                                         mybir.dt.float32, isOutput=True)

    with TileContext(nc) as tc:
        with tc.tile_pool(name="persist", bufs=1) as pp:
            idx = pp.tile([P, C, PIXROW], mybir.dt.bfloat16, tag="idx")
            acc_pe = pp.tile([P, 2 * C], mybir.dt.float32, tag="accp")
            acc_sc = pp.tile([P, C * NSC], mybir.dt.float32, tag="accs")
            acc_df = pp.tile([P, C * NDF * 4], mybir.dt.float32, tag="accd")
            bias_tab = pp.tile([P, NBINS], mybir.dt.float32, tag="bias")
            # ones-column window: zeros [P, 255] with ones in column 127.
            # lhsT = zo[:, 127-j : 255-j] puts the ones at weight column j,
            # so the matmul lands the plane's column-sums on PSUM row j.
            zo = pp.tile([P, 2 * P - 1], mybir.dt.bfloat16, tag="zo")

            nc.sync.dma_start(out=bias_tab[:], in_=bias_ext.ap())
            nc.gpsimd.memset(zo[:], 0.0)
            nc.gpsimd.memset(zo[:, P - 1:P], 1.0)

            # ---- Phase 1: prep — strided high-half copies only ----
            with tc.tile_pool(name="prep", bufs=3) as prep:
                for k in range(NCHUNK):
                    stage = prep.tile([P, CHUNK], mybir.dt.float32,
                                      tag="stage")
                    nc.sync.dma_start(
                        out=stage[:],
                        in_=x_ext.ap()[:, k * CHUNK:(k + 1) * CHUNK])
                    s16 = stage[:].bitcast(I16)      # [P, 2*CHUNK]
                    for c in range(C):
                        nc.vector.tensor_copy(
                            out=idx[:, c, k * CPIX:(k + 1) * CPIX]
                            .bitcast(I16),
                            in_=s16[:, 2 * c + 1::2 * C])

            # ---- Phase 2: count passes, three routes (all CDF) ----
            with (tc.tile_pool(name="planes", bufs=4) as plp,
                  tc.tile_pool(name="sscr", bufs=1) as ssp,
                  tc.tile_pool(name="psum", bufs=2, space="PSUM") as psp):
                for c in range(C):
                    ps0 = psp.tile([P, MMCOL], mybir.dt.float32, tag="ps0")
                    ps1 = psp.tile([P, MMCOL], mybir.dt.float32, tag="ps1")
                    ps = [ps0, ps1]

                    # interleaved emission keeps all queues fed.  DF work is
                    # split into quarter-row accums so no single DVE op
                    # stalls plane creation longer than a PE quarter-burst.
                    sched = []
                    si = NPE + 0
                    dfq = [(b, q) for b in range(NPE + NSC, NBINS)
                           for q in range(4)]
                    di = 0
                    for b in range(NPE):
                        sched.append(("pe", b, 0))
                        if b % 2 == 0 and si < NPE + NSC:
                            sched.append(("sc", si, 0)); si += 1
                        while di < len(dfq) and di * NPE < b * len(dfq):
                            sched.append(("df",) + dfq[di]); di += 1
                    while si < NPE + NSC:
                        sched.append(("sc", si, 0)); si += 1
                    while di < len(dfq):
                        sched.append(("df",) + dfq[di]); di += 1

                    mm_done = [0, 0]
                    n_mm = [min(NPE, P) * 4 * (QTR // MMCOL),
                            max(NPE - P, 0) * 4 * (QTR // MMCOL)]
                    for route, b, q in sched:
                        if route == "pe":
                            bank, brow = (0, b) if b < P else (1, b - P)
                            w = zo[:, P - 1 - brow:2 * P - 1 - brow]
                            for h in range(4):
                                plane = plp.tile([P, QTR],
                                                 mybir.dt.bfloat16,
                                                 tag="plane")
                                nc.vector.tensor_scalar(
                                    plane[:],
                                    idx[:, c, h * QTR:(h + 1) * QTR],
                                    _thresh(b), None, AL.is_ge)
                                for j in range(QTR // MMCOL):
                                    nc.tensor.matmul(
                                        ps[bank][:], w,
                                        plane[:, j * MMCOL:(j + 1) * MMCOL],
                                        start=(mm_done[bank] == 0),
                                        stop=(mm_done[bank]
                                              == n_mm[bank] - 1))
                                    mm_done[bank] += 1
                        elif route == "sc":
                            col = c * NSC + (b - NPE)
                            scr = ssp.tile([P, PIXROW], mybir.dt.bfloat16,
                                           tag="s")
                            nc.scalar.activation(
                                scr[:], idx[:, c, :], AF.Sign,
                                bias=bias_tab[:, b:b + 1], scale=1.0,
                                accum_out=acc_sc[:, col:col + 1])
                        else:
                            col = 4 * (c * NDF + (b - NPE - NSC)) + q
                            scr = ssp.tile([P, QTR], mybir.dt.bfloat16,
                                           tag="d")
                            nc.vector.tensor_scalar(
                                scr[:], idx[:, c, q * QTR:(q + 1) * QTR],
                                _thresh(b), None, AL.is_ge, AL.add,
                                accum_out=acc_df[:, col:col + 1])

                    # fold the channel's PSUM banks: bank row j = S(bin j)
                    nc.vector.tensor_reduce(
                        acc_pe[:, 2 * c:2 * c + 1], ps[0][:],
                        mybir.AxisListType.X, AL.add)
                    nc.vector.tensor_reduce(
                        acc_pe[:, 2 * c + 1:2 * c + 2], ps[1][:],
                        mybir.AxisListType.X, AL.add)

            # ---- Phase 3: results out ----
            nc.sync.dma_start(out=accp_ext.ap(), in_=acc_pe[:])
            nc.sync.dma_start(out=accs_ext.ap(), in_=acc_sc[:])
            nc.sync.dma_start(out=accd_ext.ap(), in_=acc_df[:])

    nc.finalize()
    return nc


def _get_module():
    if "nc" not in _CACHE:
        _CACHE["nc"] = _build_module()
    return _CACHE["nc"]


def _decode_counts(results):
    # S[c, b] = #{x_c >= b/256}, summed over cores; exact integers.
    S = np.zeros((C, NBINS + 1), dtype=np.float64)
    sc_sign = np.zeros((C, NSC), dtype=np.float64)
    for r in results:
        ap = r["acc_pe"].astype(np.float64)          # [P, 2C]: row j = bin j
        asc = r["acc_sc"].astype(np.float64)
        ad = r["acc_df"].astype(np.float64)
        for c in range(C):
            S[c, :P] += ap[:, 2 * c]
            S[c, P:NPE] += ap[:NPE - P, 2 * c + 1]
        sc_sign += asc.sum(axis=0).reshape(C, NSC)
        S[:, NPE + NSC:NBINS] += ad.sum(axis=0).reshape(C, NDF, 4).sum(axis=2)
    # Sign sums -> S: A[b] = 2*S(b) - TOT
    tot = float(NCORES * P * PIXROW)
    S[:, NPE:NPE + NSC] = (sc_sign + tot) / 2.0
    S[:, NBINS] = 0.0
    counts = S[:, :NBINS] - S[:, 1:]
    return counts


def run(x: np.ndarray, trace: bool = False):
    nc = _get_module()

    x = np.ascontiguousarray(x, dtype=np.float32)
    assert x.shape == (B, H, W, C)
    shards = x.reshape(NCORES, P, ROW)

    # Sign-route bias: sign(x_t + bias_b) == +1  iff  x_t >= b/256.
    # delta_b = b * 2^-18 sits strictly inside the gap below b/256.
    barr = np.arange(NBINS, dtype=np.float64)
    bias = (barr * 2.0 ** -18 - barr / 256.0).astype(np.float32)
    bias_tab = np.tile(bias[None, :], (P, 1))
    in_maps = [{"x": shards[i], "bias_tab": bias_tab} for i in range(NCORES)]

    res = run_bass_kernel_spmd(nc, in_maps, list(range(NCORES)), trace=trace)

    counts = _decode_counts(res.results)
    # Normalization exactly as the reference: fp32 divide, then transpose.
    counts32 = counts.astype(np.float32)
    sums = counts32.sum(axis=1, keepdims=True, dtype=np.float32)
    hist = counts32 / sums
    return np.ascontiguousarray(hist.T), res


def kernel(**inputs) -> np.ndarray:
    out, _ = run(inputs["inputs"],
                 trace=bool(os.environ.get("KERNEL_TRACE")))
    return out


# revision 23
# speedup vs baseline: 11.3534x; 1.0047x over previous
"""Trainium2 Bass kernel: per-channel 256-bin normalized histogram.

Input: full inputs [64, 512, 512, 3] float32 in [0, 1).
Output: [256, 3] float32 — per-channel histogram normalized to sum 1.

Strategy (8 NeuronCores, data-parallel over the batch dim):
  Each core gets 8 batches = 6,291,456 elements laid out [128, 49152]
  (partition p holds 16384 consecutive pixels, channel-interleaved).

  Key trick: bin(x) = floor(x*256) is EXACTLY determined by the
  round-toward-zero bf16 truncation of x (for x in [0,1), the integer
  part of x*256 needs at most the top 7 mantissa bits).  So prep is a
  pure byte-level copy: the high int16 half of each fp32 word,
  de-interleaved per channel (VectorE strided copies, no arithmetic).

  Counting is CDF-based: every route computes S(b) = #{x >= b/256};
  counts are recovered on the host as count[b] = S(b) - S(b+1), exact
  in integer arithmetic.  Three engine routes per channel:
    - PE route (bins [0, NPE)): VectorE builds the is_ge indicator
      plane at 4x bf16 rate; TensorE reduces it with matmuls whose
      stationary operand is a ones-column window, so bin j's S lands
      at PSUM partition j of a per-channel [128, 512] PSUM bank.  One
      tensor_reduce per bank folds it to [128, 1].
    - ScalarE route: activation(Sign, bias=eps_b - b/256, accum) over
      the raw truncated values — a CDF via sign sums.
    - DF route (VectorE fused): tensor_scalar(is_ge, accum) over
      quarter rows, interleaved so plane creation never stalls long.

  Host: sums accumulators (exact integer counts in fp64), all-reduces
  the 8 cores' counts, applies the per-channel fp32 normalize divide.

Counting is exact (integer counts < 2^24 in fp32 accumulators), so the
result matches the reference bit-for-bit up to the final fp32 divide.
"""

import os

import numpy as np

import concourse.bacc as bacc
import concourse.mybir as mybir
from concourse.bass_utils import run_bass_kernel_spmd
from concourse.tile import TileContext

# Problem constants (hardcoded per contract)
B, H, W, C = 64, 512, 512, 3
NBINS = 256
NCORES = 8
P = 128

BPC = B // NCORES                     # 8 batches per core
EPC = BPC * H * W * C                 # 6,291,456 elements per core
ROW = EPC // P                        # 49,152 fp32 per partition
PIXROW = ROW // C                     # 16,384 per channel per partition
CHUNK = 3072
NCHUNK = ROW // CHUNK                 # 16
CPIX = CHUNK // C                     # 1024

# Per-channel bin split across engine routes (sums to 256).
NPE = 155                             # bins [0, NPE) reduced on TensorE
NSC = 80                              # bins [NPE, NPE+NSC) on ScalarE
NDF = NBINS - NPE - NSC               # bins [NPE+NSC, 256) fused on VectorE

QTR = PIXROW // 4                     # PE planes are built in quarters
MMCOL = 512                           # matmul moving-columns per op

AL = mybir.AluOpType
AF = mybir.ActivationFunctionType
I16 = mybir.dt.int16

_CACHE: dict = {}


def _thresh(b: int) -> float:
    return float(np.float32(b / 256.0))


def _build_module():
    nc = bacc.Bacc("TRN2", target_bir_lowering=False, debug=False,
                   num_devices=NCORES)

    x_ext = nc.declare_dram_parameter("x", [P, ROW], mybir.dt.float32,
                                      isOutput=False)
    bias_ext = nc.declare_dram_parameter("bias_tab", [P, NBINS],
                                         mybir.dt.float32, isOutput=False)
    accp_ext = nc.declare_dram_parameter("acc_pe", [P, 2 * C],
                                         mybir.dt.float32, isOutput=True)
    accs_ext = nc.declare_dram_parameter("acc_sc", [P, C * NSC],
                                         mybir.dt.float32, isOutput=True)
    accd_ext = nc.declare_dram_parameter("acc_df", [P, C * NDF * 4],
                                         mybir.dt.float32, isOutput=True)

    with TileContext(nc) as tc:
        with tc.tile_pool(name="persist", bufs=1) as pp:
            idx = pp.tile([P, C, PIXROW], mybir.dt.bfloat16, tag="idx")
            acc_pe = pp.tile([P, 2 * C], mybir.dt.float32, tag="accp")
            acc_sc = pp.tile([P, C * NSC], mybir.dt.float32, tag="accs")
            acc_df = pp.tile([P, C * NDF * 4], mybir.dt.float32, tag="accd")
            bias_tab = pp.tile([P, NBINS], mybir.dt.float32, tag="bias")
            # ones-column window: zeros [P, 255] with ones in column 127.
            # lhsT = zo[:, 127-j : 255-j] puts the ones at weight column j,
            # so the matmul lands the plane's column-sums on PSUM row j.
            zo = pp.tile([P, 2 * P - 1], mybir.dt.bfloat16, tag="zo")

            nc.sync.dma_start(out=bias_tab[:], in_=bias_ext.ap())
            nc.gpsimd.memset(zo[:], 0.0)
            nc.gpsimd.memset(zo[:, P - 1:P], 1.0)

            # ---- Phase 1: prep — strided high-half copies only ----
            with tc.tile_pool(name="prep", bufs=3) as prep:
                for k in range(NCHUNK):
                    stage = prep.tile([P, CHUNK], mybir.dt.float32,
                                      tag="stage")
                    nc.sync.dma_start(
                        out=stage[:],
                        in_=x_ext.ap()[:, k * CHUNK:(k + 1) * CHUNK])
                    s16 = stage[:].bitcast(I16)      # [P, 2*CHUNK]
                    for c in range(C):
                        nc.vector.tensor_copy(
                            out=idx[:, c, k * CPIX:(k + 1) * CPIX]
                            .bitcast(I16),
                            in_=s16[:, 2 * c + 1::2 * C])

            # ---- Phase 2: count passes, three routes (all CDF) ----
            # One GLOBAL schedule across channels: PE bins run channel-major
            # (PSUM bank per channel-half), while ScalarE bins and the DVE
            # fused (DF) quarter-accums are paced uniformly against total PE
            # progress so no engine idles at the tail.
            with (tc.tile_pool(name="planes", bufs=6) as plp,
                  tc.tile_pool(name="sscr", bufs=1) as ssp,
                  tc.tile_pool(name="psum", bufs=2, space="PSUM") as psp):
                sc_items = [(c, b) for c in range(C)
                            for b in range(NPE, NPE + NSC)]
                df_items = [(c, b, q) for c in range(C)
                            for b in range(NPE + NSC, NBINS)
                            for q in range(4)]
                n_pe_tot = C * NPE
                si = di = 0

                for c in range(C):
                    ps0 = psp.tile([P, MMCOL], mybir.dt.float32, tag="ps0")
                    ps1 = psp.tile([P, MMCOL], mybir.dt.float32, tag="ps1")
                    ps = [ps0, ps1]
                    mm_done = [0, 0]
                    n_mm = [min(NPE, P) * 4 * (QTR // MMCOL),
                            max(NPE - P, 0) * 4 * (QTR // MMCOL)]

                    for b in range(NPE):
                        t = c * NPE + b
                        # PE bin: 4 quarter planes + 32 matmuls
                        bank, brow = (0, b) if b < P else (1, b - P)
                        w = zo[:, P - 1 - brow:2 * P - 1 - brow]
                        for h in range(4):
                            plane = plp.tile([P, QTR], mybir.dt.bfloat16,
                                             tag="plane")
                            nc.vector.tensor_scalar(
                                plane[:],
                                idx[:, c, h * QTR:(h + 1) * QTR],
                                _thresh(b), None, AL.is_ge)
                            for j in range(QTR // MMCOL):
                                nc.tensor.matmul(
                                    ps[bank][:], w,
                                    plane[:, j * MMCOL:(j + 1) * MMCOL],
                                    start=(mm_done[bank] == 0),
                                    stop=(mm_done[bank] == n_mm[bank] - 1))
                                mm_done[bank] += 1
                        # paced ScalarE bins
                        while (si < len(sc_items)
                               and si * n_pe_tot <= t * len(sc_items)):
                            sc_c, sc_b = sc_items[si]; si += 1
                            col = sc_c * NSC + (sc_b - NPE)
                            scr = ssp.tile([P, PIXROW], mybir.dt.bfloat16,
                                           tag="s")
                            nc.scalar.activation(
                                scr[:], idx[:, sc_c, :], AF.Sign,
                                bias=bias_tab[:, sc_b:sc_b + 1], scale=1.0,
                                accum_out=acc_sc[:, col:col + 1])
                        # paced DVE fused quarter-accums (skip the PE
                        # warm-up window at the very start)
                        while (di < len(df_items) and t >= 4
                               and di * (n_pe_tot - 4)
                               <= (t - 4) * len(df_items)):
                            df_c, df_b, q = df_items[di]; di += 1
                            col = 4 * (df_c * NDF + (df_b - NPE - NSC)) + q
                            scr = ssp.tile([P, QTR], mybir.dt.bfloat16,
                                           tag="d")
                            nc.vector.tensor_scalar(
                                scr[:], idx[:, df_c, q * QTR:(q + 1) * QTR],
                                _thresh(df_b), None, AL.is_ge, AL.add,
                                accum_out=acc_df[:, col:col + 1])

                    # fold the channel's PSUM banks: bank row j = S(bin j)
                    nc.vector.tensor_reduce(
                        acc_pe[:, 2 * c:2 * c + 1], ps[0][:],
                        mybir.AxisListType.X, AL.add)
                    nc.vector.tensor_reduce(
                        acc_pe[:, 2 * c + 1:2 * c + 2], ps[1][:],
                        mybir.AxisListType.X, AL.add)

                while si < len(sc_items):
                    sc_c, sc_b = sc_items[si]; si += 1
                    col = sc_c * NSC + (sc_b - NPE)
                    scr = ssp.tile([P, PIXROW], mybir.dt.bfloat16, tag="s")
                    nc.scalar.activation(
                        scr[:], idx[:, sc_c, :], AF.Sign,
                        bias=bias_tab[:, sc_b:sc_b + 1], scale=1.0,
                        accum_out=acc_sc[:, col:col + 1])
                while di < len(df_items):
                    df_c, df_b, q = df_items[di]; di += 1
                    col = 4 * (df_c * NDF + (df_b - NPE - NSC)) + q
                    scr = ssp.tile([P, QTR], mybir.dt.bfloat16, tag="d")
                    nc.vector.tensor_scalar(
                        scr[:], idx[:, df_c, q * QTR:(q + 1) * QTR],
                        _thresh(df_b), None, AL.is_ge, AL.add,
                        accum_out=acc_df[:, col:col + 1])

            # ---- Phase 3: results out ----
            nc.sync.dma_start(out=accp_ext.ap(), in_=acc_pe[:])
            nc.sync.dma_start(out=accs_ext.ap(), in_=acc_sc[:])
            nc.sync.dma_start(out=accd_ext.ap(), in_=acc_df[:])

    nc.finalize()
    return nc


def _get_module():
    if "nc" not in _CACHE:
        _CACHE["nc"] = _build_module()
    return _CACHE["nc"]


def _decode_counts(results):
    # S[c, b] = #{x_c >= b/256}, summed over cores; exact integers.
    S = np.zeros((C, NBINS + 1), dtype=np.float64)
    sc_sign = np.zeros((C, NSC), dtype=np.float64)
    for r in results:
        ap = r["acc_pe"].astype(np.float64)          # [P, 2C]: row j = bin j
        asc = r["acc_sc"].astype(np.float64)
        ad = r["acc_df"].astype(np.float64)
        for c in range(C):
            S[c, :P] += ap[:, 2 * c]
            S[c, P:NPE] += ap[:NPE - P, 2 * c + 1]
        sc_sign += asc.sum(axis=0).reshape(C, NSC)
        S[:, NPE + NSC:NBINS] += ad.sum(axis=0).reshape(C, NDF, 4).sum(axis=2)
    # Sign sums -> S: A[b] = 2*S(b) - TOT
    tot = float(NCORES * P * PIXROW)
    S[:, NPE:NPE + NSC] = (sc_sign + tot) / 2.0
    S[:, NBINS] = 0.0
    counts = S[:, :NBINS] - S[:, 1:]
    return counts


def run(x: np.ndarray, trace: bool = False):
    nc = _get_module()

    x = np.ascontiguousarray(x, dtype=np.float32)
    assert x.shape == (B, H, W, C)
    shards = x.reshape(NCORES, P, ROW)

    # Sign-route bias: sign(x_t + bias_b) == +1  iff  x_t >= b/256.
    # delta_b = b * 2^-18 sits strictly inside the gap below b/256.
    barr = np.arange(NBINS, dtype=np.float64)
    bias = (barr * 2.0 ** -18 - barr / 256.0).astype(np.float32)
    bias_tab = np.tile(bias[None, :], (P, 1))
    in_maps = [{"x": shards[i], "bias_tab": bias_tab} for i in range(NCORES)]

    res = run_bass_kernel_spmd(nc, in_maps, list(range(NCORES)), trace=trace)

    counts = _decode_counts(res.results)
    # Normalization exactly as the reference: fp32 divide, then transpose.
    counts32 = counts.astype(np.float32)
    sums = counts32.sum(axis=1, keepdims=True, dtype=np.float32)
    hist = counts32 / sums
    return np.ascontiguousarray(hist.T), res


def kernel(**inputs) -> np.ndarray:
    out, _ = run(inputs["inputs"],
                 trace=bool(os.environ.get("KERNEL_TRACE")))
    return out


# revision 25
# speedup vs baseline: 13.3287x; 1.1740x over previous
"""Trainium2 Bass kernel: per-channel 256-bin normalized histogram.

Input: full inputs [64, 512, 512, 3] float32 in [0, 1).
Output: [256, 3] float32 — per-channel histogram normalized to sum 1.

Strategy (8 NeuronCores, data-parallel over the batch dim):
  Each core gets 8 batches = 6,291,456 elements laid out [128, 49152]
  (partition p holds 16384 consecutive pixels, channel-interleaved).

  Key trick: bin(x) = floor(x*256) is EXACTLY determined by the
  round-toward-zero bf16 truncation of x (for x in [0,1), the integer
  part of x*256 needs at most the top 7 mantissa bits).  So prep is a
  pure byte-level copy: the high int16 half of each fp32 word,
  de-interleaved per channel (VectorE strided copies, no arithmetic).

  Counting is CDF-based: every route computes S(b) = #{x >= b/256};
  counts are recovered on the host as count[b] = S(b) - S(b+1), exact
  in integer arithmetic.  Three engine routes per channel:
    - PE route (bins [0, NPE)): VectorE builds the is_ge indicator
      plane at 4x bf16 rate; TensorE reduces it with matmuls whose
      stationary operand is a ones-column window, so bin j's S lands
      at PSUM partition j of a per-channel [128, 512] PSUM bank.  One
      tensor_reduce per bank folds it to [128, 1].
    - ScalarE route: activation(Sign, bias=eps_b - b/256, accum) over
      the raw truncated values — a CDF via sign sums.
    - DF route (VectorE fused): tensor_scalar(is_ge, accum) over
      quarter rows, interleaved so plane creation never stalls long.

  Host: sums accumulators (exact integer counts in fp64), all-reduces
  the 8 cores' counts, applies the per-channel fp32 normalize divide.

Counting is exact (integer counts < 2^24 in fp32 accumulators), so the
result matches the reference bit-for-bit up to the final fp32 divide.
"""

import os

import numpy as np

import concourse.bacc as bacc
import concourse.mybir as mybir
from concourse.bass_utils import run_bass_kernel_spmd
from concourse.tile import TileContext

# Problem constants (hardcoded per contract)
B, H, W, C = 64, 512, 512, 3
NBINS = 256
NCORES = 8
P = 128

BPC = B // NCORES                     # 8 batches per core
EPC = BPC * H * W * C                 # 6,291,456 elements per core
ROW = EPC // P                        # 49,152 fp32 per partition
PIXROW = ROW // C                     # 16,384 per channel per partition
CHUNK = 3072
NCHUNK = ROW // CHUNK                 # 16
CPIX = CHUNK // C                     # 1024

# Per-channel bin split across engine routes (sums to 256).
NPE = 155                             # bins [0, NPE) reduced on TensorE
NSC = 80                              # bins [NPE, NPE+NSC) on ScalarE
NDF = NBINS - NPE - NSC               # bins [NPE+NSC, 256) fused on VectorE

QTR = PIXROW // 4                     # PE planes are built in quarters
MMCOL = 512                           # matmul moving-columns per op

AL = mybir.AluOpType
AF = mybir.ActivationFunctionType
I16 = mybir.dt.int16

_CACHE: dict = {}


def _thresh(b: int) -> float:
    return float(np.float32(b / 256.0))


def _build_module():
    nc = bacc.Bacc("TRN2", target_bir_lowering=False, debug=False,
                   num_devices=NCORES)

    x_ext = nc.declare_dram_parameter("x", [P, ROW], mybir.dt.float32,
                                      isOutput=False)
    bias_ext = nc.declare_dram_parameter("bias_tab", [P, NBINS],
                                         mybir.dt.float32, isOutput=False)
    accp_ext = nc.declare_dram_parameter("acc_pe", [P, 2 * C],
                                         mybir.dt.float32, isOutput=True)
    accs_ext = nc.declare_dram_parameter("acc_sc", [P, C * NSC],
                                         mybir.dt.float32, isOutput=True)
    accd_ext = nc.declare_dram_parameter("acc_df", [P, C * NDF * 4],
                                         mybir.dt.float32, isOutput=True)

    with TileContext(nc) as tc:
        with tc.tile_pool(name="persist", bufs=1) as pp:
            idx = pp.tile([P, C, PIXROW], mybir.dt.bfloat16, tag="idx")
            acc_pe = pp.tile([P, 2 * C], mybir.dt.float32, tag="accp")
            acc_sc = pp.tile([P, C * NSC], mybir.dt.float32, tag="accs")
            acc_df = pp.tile([P, C * NDF * 4], mybir.dt.float32, tag="accd")
            bias_tab = pp.tile([P, NBINS], mybir.dt.float32, tag="bias")
            # ones-column window: zeros [P, 255] with ones in column 127.
            # lhsT = zo[:, 127-j : 255-j] puts the ones at weight column j,
            # so the matmul lands the plane's column-sums on PSUM row j.
            zo = pp.tile([P, 2 * P - 1], mybir.dt.bfloat16, tag="zo")

            nc.sync.dma_start(out=bias_tab[:], in_=bias_ext.ap())
            nc.gpsimd.memset(zo[:], 0.0)
            nc.gpsimd.memset(zo[:, P - 1:P], 1.0)

            # ---- Phase 1: prep — strided high-half copies only ----
            with tc.tile_pool(name="prep", bufs=3) as prep:
                for k in range(NCHUNK):
                    stage = prep.tile([P, CHUNK], mybir.dt.float32,
                                      tag="stage")
                    nc.sync.dma_start(
                        out=stage[:],
                        in_=x_ext.ap()[:, k * CHUNK:(k + 1) * CHUNK])
                    s16 = stage[:].bitcast(I16)      # [P, 2*CHUNK]
                    for c in range(C):
                        nc.vector.tensor_copy(
                            out=idx[:, c, k * CPIX:(k + 1) * CPIX]
                            .bitcast(I16),
                            in_=s16[:, 2 * c + 1::2 * C])

            # ---- Phase 2: count passes, three routes (all CDF) ----
            # One GLOBAL schedule across channels: PE bins run channel-major
            # (PSUM bank per channel-half), while ScalarE bins and the DVE
            # fused (DF) quarter-accums are paced uniformly against total PE
            # progress so no engine idles at the tail.
            with (tc.tile_pool(name="planes", bufs=6) as plp,
                  tc.tile_pool(name="sscr", bufs=1) as ssp,
                  tc.tile_pool(name="psum", bufs=2, space="PSUM") as psp):
                sc_items = [(c, b) for c in range(C)
                            for b in range(NPE, NPE + NSC)]
                df_items = [(c, b, q) for c in range(C)
                            for b in range(NPE + NSC, NBINS)
                            for q in range(4)]
                n_pe_tot = C * NPE
                si = di = 0

                for c in range(C):
                    ps0 = psp.tile([P, MMCOL], mybir.dt.float32, tag="ps0")
                    ps1 = psp.tile([P, MMCOL], mybir.dt.float32, tag="ps1")
                    ps = [ps0, ps1]
                    mm_done = [0, 0]
                    n_mm = [min(NPE, P) * 4 * (QTR // MMCOL),
                            max(NPE - P, 0) * 4 * (QTR // MMCOL)]

                    for b in range(NPE):
                        t = c * NPE + b
                        # PE bin: 4 quarter planes + 32 matmuls
                        bank, brow = (0, b) if b < P else (1, b - P)
                        w = zo[:, P - 1 - brow:2 * P - 1 - brow]
                        for h in range(4):
                            plane = plp.tile([P, QTR], mybir.dt.bfloat16,
                                             tag="plane")
                            nc.vector.tensor_scalar(
                                plane[:],
                                idx[:, c, h * QTR:(h + 1) * QTR],
                                _thresh(b), None, AL.is_ge)
                            for j in range(QTR // MMCOL):
                                nc.tensor.matmul(
                                    ps[bank][:], w,
                                    plane[:, j * MMCOL:(j + 1) * MMCOL],
                                    start=(mm_done[bank] == 0),
                                    stop=(mm_done[bank] == n_mm[bank] - 1))
                                mm_done[bank] += 1
                        # paced ScalarE bins
                        while (si < len(sc_items)
                               and si * n_pe_tot <= t * len(sc_items)):
                            sc_c, sc_b = sc_items[si]; si += 1
                            col = sc_c * NSC + (sc_b - NPE)
                            scr = ssp.tile([P, PIXROW], mybir.dt.bfloat16,
                                           tag="s")
                            nc.scalar.activation(
                                scr[:], idx[:, sc_c, :], AF.Sign,
                                bias=bias_tab[:, sc_b:sc_b + 1], scale=1.0,
                                accum_out=acc_sc[:, col:col + 1])
                        # paced DVE fused quarter-accums (skip the PE
                        # warm-up window at the very start)
                        while (di < len(df_items) and t >= 4
                               and di * (n_pe_tot - 4)
                               <= (t - 4) * len(df_items)):
                            df_c, df_b, q = df_items[di]; di += 1
                            col = 4 * (df_c * NDF + (df_b - NPE - NSC)) + q
                            # allocate from the plane pool: the WAR dep on
                            # a recent plane's matmuls locksteps DF work to
                            # PE progress (the Tile scheduler reorders free
                            # ops arbitrarily otherwise)
                            scr = plp.tile([P, QTR], mybir.dt.bfloat16,
                                           tag="plane")
                            nc.vector.tensor_scalar(
                                scr[:], idx[:, df_c, q * QTR:(q + 1) * QTR],
                                _thresh(df_b), None, AL.is_ge, AL.add,
                                accum_out=acc_df[:, col:col + 1])

                    # fold the channel's PSUM banks: bank row j = S(bin j)
                    nc.vector.tensor_reduce(
                        acc_pe[:, 2 * c:2 * c + 1], ps[0][:],
                        mybir.AxisListType.X, AL.add)
                    nc.vector.tensor_reduce(
                        acc_pe[:, 2 * c + 1:2 * c + 2], ps[1][:],
                        mybir.AxisListType.X, AL.add)

                while si < len(sc_items):
                    sc_c, sc_b = sc_items[si]; si += 1
                    col = sc_c * NSC + (sc_b - NPE)
                    scr = ssp.tile([P, PIXROW], mybir.dt.bfloat16, tag="s")
                    nc.scalar.activation(
                        scr[:], idx[:, sc_c, :], AF.Sign,
                        bias=bias_tab[:, sc_b:sc_b + 1], scale=1.0,
                        accum_out=acc_sc[:, col:col + 1])
                while di < len(df_items):
                    df_c, df_b, q = df_items[di]; di += 1
                    col = 4 * (df_c * NDF + (df_b - NPE - NSC)) + q
                    scr = plp.tile([P, QTR], mybir.dt.bfloat16, tag="plane")
                    nc.vector.tensor_scalar(
                        scr[:], idx[:, df_c, q * QTR:(q + 1) * QTR],
                        _thresh(df_b), None, AL.is_ge, AL.add,
                        accum_out=acc_df[:, col:col + 1])

            # ---- Phase 3: results out ----
            nc.sync.dma_start(out=accp_ext.ap(), in_=acc_pe[:])
            nc.sync.dma_start(out=accs_ext.ap(), in_=acc_sc[:])
            nc.sync.dma_start(out=accd_ext.ap(), in_=acc_df[:])

    nc.finalize()
    return nc


def _get_module():
    if "nc" not in _CACHE:
        _CACHE["nc"] = _build_module()
    return _CACHE["nc"]


def _decode_counts(results):
    # S[c, b] = #{x_c >= b/256}, summed over cores; exact integers.
    S = np.zeros((C, NBINS + 1), dtype=np.float64)
    sc_sign = np.zeros((C, NSC), dtype=np.float64)
    for r in results:
        ap = r["acc_pe"].astype(np.float64)          # [P, 2C]: row j = bin j
        asc = r["acc_sc"].astype(np.float64)
        ad = r["acc_df"].astype(np.float64)
        for c in range(C):
            S[c, :P] += ap[:, 2 * c]
            S[c, P:NPE] += ap[:NPE - P, 2 * c + 1]
        sc_sign += asc.sum(axis=0).reshape(C, NSC)
        S[:, NPE + NSC:NBINS] += ad.sum(axis=0).reshape(C, NDF, 4).sum(axis=2)
    # Sign sums -> S: A[b] = 2*S(b) - TOT
    tot = float(NCORES * P * PIXROW)
    S[:, NPE:NPE + NSC] = (sc_sign + tot) / 2.0
    S[:, NBINS] = 0.0
    counts = S[:, :NBINS] - S[:, 1:]
    return counts


def run(x: np.ndarray, trace: bool = False):
    nc = _get_module()

    x = np.ascontiguousarray(x, dtype=np.float32)
    assert x.shape == (B, H, W, C)
    shards = x.reshape(NCORES, P, ROW)

    # Sign-route bias: sign(x_t + bias_b) == +1  iff  x_t >= b/256.
    # delta_b = b * 2^-18 sits strictly inside the gap below b/256.
    barr = np.arange(NBINS, dtype=np.float64)
    bias = (barr * 2.0 ** -18 - barr / 256.0).astype(np.float32)
    bias_tab = np.tile(bias[None, :], (P, 1))
    in_maps = [{"x": shards[i], "bias_tab": bias_tab} for i in range(NCORES)]

    res = run_bass_kernel_spmd(nc, in_maps, list(range(NCORES)), trace=trace)

    counts = _decode_counts(res.results)
    # Normalization exactly as the reference: fp32 divide, then transpose.
    counts32 = counts.astype(np.float32)
    sums = counts32.sum(axis=1, keepdims=True, dtype=np.float32)
    hist = counts32 / sums
    return np.ascontiguousarray(hist.T), res


def kernel(**inputs) -> np.ndarray:
    out, _ = run(inputs["inputs"],
                 trace=bool(os.environ.get("KERNEL_TRACE")))
    return out
